# revision 2
# baseline (speedup 1.0000x reference)
"""Trainium2 Bass kernel for nn_DistanceProbeAlternative (retrieval_knn).

Computes, per batch b:
    proj = emb[b] @ W.T                      # [S, R]
    dist[i, j] = ||proj_i||^2 - 2 proj_i . proj_j + ||proj_j||^2

Sharding: data-parallel over batch B=32 across 8 cores (4 batches/core).
W is replicated. No collectives.

v5 design (DMA-roofline oriented; ~13MB HBM traffic/core @ ~358GB/s):
  * Host lays out emb as embP16 [b, p, k, s] (p = d%128, k = d//128) so
    every input DMA moves 128 partitions x multi-KB contiguous lines
    (big descriptors, HWDGE line rate). k-split chunks let the PE start
    partial-k proj accumulation as soon as the first chunk lands.
  * Output is PACKED: the 8 upper-triangle block-rows of a batch are
    concatenated into one [128, 4608] fp16 SBUF tile and written with 2
    large contiguous DMAs per batch. The host unpacks + mirrors.
  * All DMA on the sync HWDGE ring: input triggers first (FIFO => input
    has priority; PE is fed as early as possible), output drains after.
  * No projTm2: dots matmuls produce +dot; the -2 rides the epilogue op
    (ACT scale=-2 bias=+n_i, or DVE tensor_scalar mult/add). sq is fp16
    so all norm matmuls run at full fp16 PE rate.
  * Epilogue per 512-chunk: {ACT activation | DVE tensor_scalar} into a
    tmp, then fp16 add of rowrep (n_j) on {DVE | GPSIMD}. Paths are
    spread across engines so the last batch's epilogue drains in
    parallel (short tail).
"""

import numpy as np
from contextlib import ExitStack

import concourse.bass as bass
import concourse.bacc as bacc
import concourse.tile as tile
from concourse import mybir
from concourse.bass_utils import run_bass_kernel_spmd

B, S, D, R = 32, 1024, 1024, 128
NCORES = 8
BPC = B // NCORES  # batches per core
NDT = D // 128     # 8 d-blocks
NST = S // 128     # 8 i-tiles

F32 = mybir.dt.float32
F16 = mybir.dt.float16
IDENT = mybir.ActivationFunctionType.Identity
ADD = mybir.AluOpType.add
MULT = mybir.AluOpType.mult

# packed output column offsets: tile i occupies [OFFS[i], OFFS[i]+Wi)
WIDTHS = [S - 128 * i for i in range(NST)]
OFFS = [0]
for w in WIDTHS[:-1]:
    OFFS.append(OFFS[-1] + w)
TOT = OFFS[-1] + WIDTHS[-1]  # 4608

# input k-chunking per batch (k-blocks per DMA chunk)
CHUNKS = [[2, 2, 2, 2], [4, 4], [4, 4], [4, 4]]

# output drain split (packed columns)
OUT_SPLIT = OFFS[4]  # 3328: tiles 0-3 | tiles 4-7


def build_nc():
    nc = bacc.Bacc("TRN2", target_bir_lowering=False, debug=False)

    embPd = nc.dram_tensor("embP16", [BPC, 128, NDT, S], F16, kind="ExternalInput")
    WTd = nc.dram_tensor("WT16", [128, D], F16, kind="ExternalInput")
    outPd = nc.dram_tensor("outP16", [BPC, 128, TOT], F16, kind="ExternalOutput")

    with tile.TileContext(nc) as tc, ExitStack() as ctx:
        constp = ctx.enter_context(tc.tile_pool(name="const", bufs=1))
        embT_p = ctx.enter_context(tc.tile_pool(name="embT", bufs=BPC))
        projT_p = ctx.enter_context(tc.tile_pool(name="projT", bufs=2))
        sq_p = ctx.enter_context(tc.tile_pool(name="sq", bufs=2))
        ncol_p = ctx.enter_context(tc.tile_pool(name="ncol", bufs=2))
        rowrep_p = ctx.enter_context(tc.tile_pool(name="rowrep", bufs=2))
        out_p = ctx.enter_context(tc.tile_pool(name="outsb", bufs=BPC))
        tmp_p = ctx.enter_context(tc.tile_pool(name="tmpsb", bufs=4))
        projps_p = ctx.enter_context(tc.tile_pool(name="projps", bufs=2, space="PSUM"))
        dotps_p = ctx.enter_context(tc.tile_pool(name="dotps", bufs=4, space="PSUM"))
        normps_p = ctx.enter_context(tc.tile_pool(name="normps", bufs=2, space="PSUM"))

        WT16 = constp.tile([128, D], F16, name="WT16")
        nc.sync.dma_start(out=WT16, in_=WTd.ap())

        ones16 = constp.tile([128, 128], F16, name="ones16")
        nc.vector.memset(ones16, 1.0)

        # ---- all input DMA triggers up front on the sync HWDGE ring ----
        embTs = []
        for b in range(BPC):
            embT = embT_p.tile([128, NDT * S], F16, name="embT")
            embTs.append(embT)
            dst = embT.rearrange("p (k s) -> p k s", k=NDT)
            src = embPd.ap()[b]
            k0 = 0
            for ck in CHUNKS[b]:
                nc.sync.dma_start(
                    out=dst[:, k0 : k0 + ck, :], in_=src[:, k0 : k0 + ck, :]
                )
                k0 += ck

        def proj_batch(b):
            """projT fp16 [128, S] + sq fp16, accumulated k-chunk-wise."""
            embT = embTs[b]
            projT = projT_p.tile([128, S], F16, name="projT")
            sq = sq_p.tile([128, S], F16, name="sq")
            pps = [
                projps_p.tile([128, 512], F32, name="projps") for _ in range(2)
            ]
            k0 = 0
            for ck in CHUNKS[b]:
                for k in range(k0, k0 + ck):
                    for h in range(2):
                        nc.tensor.matmul(
                            pps[h],
                            WT16[:, 128 * k : 128 * (k + 1)],
                            embT[:, S * k + 512 * h : S * k + 512 * (h + 1)],
                            start=(k == 0),
                            stop=(k == NDT - 1),
                        )
                k0 += ck
            for h in range(2):
                sl = slice(512 * h, 512 * (h + 1))
                nc.vector.tensor_copy(projT[:, sl], pps[h])
                nc.vector.tensor_mul(sq[:, sl], projT[:, sl], projT[:, sl])
            return projT, sq

        def norms_batch(sq):
            """ncol f32 [128, 2/i-tile] (n_i per-partition) and
            rowrep fp16 [128, S] (n_j on every partition)."""
            ncol_ps = normps_p.tile([128, 16], F32, tag="np", name="ncol_ps")
            for i in range(NST):
                nc.tensor.matmul(
                    ncol_ps[:, 2 * i : 2 * i + 2],
                    sq[:, 128 * i : 128 * (i + 1)],
                    ones16[:, 0:2],
                    start=True,
                    stop=True,
                )
            ncol = ncol_p.tile([128, 2 * NST], F32, name="ncol")
            nc.vector.tensor_copy(ncol, ncol_ps)

            rowrep = rowrep_p.tile([128, S], F16, name="rowrep")
            for h in range(2):
                rp = normps_p.tile([128, 512], F32, tag="np", name="rp_ps")
                nc.tensor.matmul(
                    rp, ones16, sq[:, 512 * h : 512 * (h + 1)],
                    start=True, stop=True,
                )
                nc.scalar.copy(rowrep[:, 512 * h : 512 * (h + 1)], rp)
            return ncol, rowrep

        def dots_batch(b, projT, ncol, rowrep):
            """Upper-triangle tiles into packed outsb; 2 out-DMAs."""
            outsb = out_p.tile([128, TOT], F16, name="outsb")
            nchunk = 0
            for i in range(NST):
                j0 = 128 * i
                Wi = WIDTHS[i]
                off = OFFS[i]
                pos = 0
                while pos < Wi:
                    w = min(512, Wi - pos)
                    d_ps = dotps_p.tile([128, w], F32, tag="dp", name="d_ps")
                    nc.tensor.matmul(
                        d_ps,
                        projT[:, j0 : j0 + 128],
                        projT[:, j0 + pos : j0 + pos + w],
                        start=True,
                        stop=True,
                    )
                    o = outsb[:, off + pos : off + pos + w]
                    rr = rowrep[:, j0 + pos : j0 + pos + w]
                    nb = ncol[:, 2 * i : 2 * i + 1]
                    tmp = tmp_p.tile([128, 512], F16, name="tmp")[:, 0:w]
                    # -2*dot + n_i: spread across ACT / DVE
                    if nchunk % 3 == 2:
                        nc.vector.tensor_scalar(tmp, d_ps, -2.0, nb, MULT, ADD)
                    else:
                        nc.scalar.activation(tmp, d_ps, IDENT, bias=nb, scale=-2.0)
                    # + n_j: spread DVE / GPSIMD
                    if nchunk % 2 == 0:
                        nc.vector.tensor_add(o, tmp, rr)
                    else:
                        nc.gpsimd.tensor_add(o, tmp, rr)
                    nchunk += 1
                    pos += w
                if i == 3:
                    nc.sync.dma_start(
                        out=outPd.ap()[b, :, 0:OUT_SPLIT],
                        in_=outsb[:, 0:OUT_SPLIT],
                    )
            nc.sync.dma_start(
                out=outPd.ap()[b, :, OUT_SPLIT:TOT],
                in_=outsb[:, OUT_SPLIT:TOT],
            )

        for b in range(BPC):
            projT, sq = proj_batch(b)
            ncol, rowrep = norms_batch(sq)
            dots_batch(b, projT, ncol, rowrep)

    nc.finalize()
    return nc


_NC_CACHE = None


def _get_nc():
    global _NC_CACHE
    if _NC_CACHE is None:
        _NC_CACHE = build_nc()
    return _NC_CACHE


def _host_wt16(W):
    # WT16[p, 128k + j] = W[j, 128k + p]  (W^T in [d-part, k, r] blocks)
    Wf = np.asarray(W, dtype=np.float32)
    wt = Wf.T.reshape(NDT, 128, 128).transpose(1, 0, 2).reshape(128, D)
    return np.ascontiguousarray(wt).astype(np.float16)


def _host_embp(emb16_core):
    # embP[b, p, k, s] = emb16[b, s, 128k + p]
    return np.ascontiguousarray(
        emb16_core.reshape(BPC, S, NDT, 128).transpose(0, 3, 2, 1)
    )


def run(embeddings_batch, W, trace=False, tmpdir=None):
    nc = _get_nc()
    emb16 = np.asarray(embeddings_batch, dtype=np.float32).astype(np.float16)
    wt16 = _host_wt16(W)
    in_maps = [
        {
            "embP16": _host_embp(emb16[c * BPC : (c + 1) * BPC]),
            "WT16": wt16,
        }
        for c in range(NCORES)
    ]
    res = run_bass_kernel_spmd(
        nc, in_maps, core_ids=list(range(NCORES)), trace=trace, tmpdir=tmpdir
    )
    # unpack: outP16 [BPC, 128, TOT] -> dist blocks j >= i; mirror the rest
    full = np.empty((B, S, S), dtype=np.float16)
    for c in range(NCORES):
        P = res.results[c]["outP16"]
        for b in range(BPC):
            g = c * BPC + b
            for i in range(NST):
                full[g, 128 * i : 128 * (i + 1), 128 * i : S] = P[
                    b, :, OFFS[i] : OFFS[i] + WIDTHS[i]
                ]
    NB = NST
    M = full.reshape(B, NB, 128, NB, 128)
    iu = np.triu_indices(NB, 1)
    M[:, iu[1], :, iu[0], :] = M[:, iu[0], :, iu[1], :].swapaxes(-1, -2)
    return full.astype(np.float32), res


def kernel(embeddings_batch, W):
    full, _ = run(embeddings_batch, W, trace=False)
    return full


# revision 3
# speedup vs baseline: 1.0044x; 1.0044x over previous
"""Trainium2 Bass kernel for nn_DistanceProbeAlternative (retrieval_knn).

Computes, per batch b:
    proj = emb[b] @ W.T                      # [S, R]
    dist[i, j] = ||proj_i||^2 - 2 proj_i . proj_j + ||proj_j||^2

Sharding: data-parallel over batch B=32 across 8 cores (4 batches/core).
W is replicated. No collectives.

v6 design (DMA-roofline oriented; ~13MB HBM traffic/core @ ~358GB/s):
  * Host lays out emb as embP16 [b, p, k, s] (p = d%128, k = d//128) so
    every input DMA moves 128 partitions x multi-KB contiguous lines
    (big descriptors, HWDGE line rate). k-split chunks let the PE start
    partial-k proj accumulation as soon as the first chunk lands.
  * Output is PACKED: the 8 upper-triangle block-rows of a batch are
    concatenated into one [128, 4608] fp16 SBUF tile, written with 3
    contiguous DMAs per batch as tiles complete. Host unpacks+mirrors.
  * All DMA on the sync HWDGE ring: input triggers first (FIFO => input
    has priority; PE is fed as early as possible), output drains after.
  * PE warm-up: dummy matmuls during the input DMA dead-time hold the
    HAM clock gate at full rate before the first real matmul.
  * PE emission interleaves proj(b+1) chunks inside dots(b) so the PE
    never idles (no HAM re-throttle mid-kernel).
  * No projTm2: dots matmuls produce +dot; the -2 rides the epilogue
    (ACT scale=-2 bias=+n_i, or DVE tensor_scalar). sq is fp16 so all
    norm matmuls run at full fp16 PE rate. Epilogue add of rowrep
    (+n_j) is spread across DVE/GPSIMD; the last batch's final tiles
    avoid GPSIMD (shortest tail).
"""

import numpy as np
from contextlib import ExitStack

import concourse.bass as bass
import concourse.bacc as bacc
import concourse.tile as tile
from concourse import mybir
from concourse.bass_utils import run_bass_kernel_spmd

B, S, D, R = 32, 1024, 1024, 128
NCORES = 8
BPC = B // NCORES  # batches per core
NDT = D // 128     # 8 d-blocks
NST = S // 128     # 8 i-tiles

F32 = mybir.dt.float32
F16 = mybir.dt.float16
IDENT = mybir.ActivationFunctionType.Identity
ADD = mybir.AluOpType.add
MULT = mybir.AluOpType.mult

# packed output column offsets: tile i occupies [OFFS[i], OFFS[i]+Wi)
WIDTHS = [S - 128 * i for i in range(NST)]
OFFS = [0]
for w in WIDTHS[:-1]:
    OFFS.append(OFFS[-1] + w)
TOT = OFFS[-1] + WIDTHS[-1]  # 4608

# input k-chunking per batch (k-blocks per DMA chunk)
CHUNKS = [[1, 1, 2, 4], [4, 4], [4, 4], [4, 4]]

# output drain split (packed columns): after tile 1, tile 3, tile 7
OUT_CUTS = [(1, 0, OFFS[2]), (3, OFFS[2], OFFS[4]), (7, OFFS[4], TOT)]

N_WARM = 26  # PE warm-up dummy matmuls


def build_nc():
    nc = bacc.Bacc("TRN2", target_bir_lowering=False, debug=False)

    embPd = nc.dram_tensor("embP16", [BPC, 128, NDT, S], F16, kind="ExternalInput")
    WTd = nc.dram_tensor("WT16", [128, D], F16, kind="ExternalInput")
    outPd = nc.dram_tensor("outP16", [BPC, 128, TOT], F16, kind="ExternalOutput")

    with tile.TileContext(nc) as tc, ExitStack() as ctx:
        constp = ctx.enter_context(tc.tile_pool(name="const", bufs=1))
        embT_p = ctx.enter_context(tc.tile_pool(name="embT", bufs=BPC))
        projT_p = ctx.enter_context(tc.tile_pool(name="projT", bufs=2))
        sq_p = ctx.enter_context(tc.tile_pool(name="sq", bufs=2))
        ncol_p = ctx.enter_context(tc.tile_pool(name="ncol", bufs=2))
        rowrep_p = ctx.enter_context(tc.tile_pool(name="rowrep", bufs=2))
        out_p = ctx.enter_context(tc.tile_pool(name="outsb", bufs=BPC))
        tmp_p = ctx.enter_context(tc.tile_pool(name="tmpsb", bufs=4))
        projps_p = ctx.enter_context(tc.tile_pool(name="projps", bufs=2, space="PSUM"))
        dotps_p = ctx.enter_context(tc.tile_pool(name="dotps", bufs=3, space="PSUM"))
        normps_p = ctx.enter_context(tc.tile_pool(name="normps", bufs=2, space="PSUM"))
        warmps_p = ctx.enter_context(tc.tile_pool(name="warmps", bufs=1, space="PSUM"))

        WT16 = constp.tile([128, D], F16, name="WT16")
        nc.sync.dma_start(out=WT16, in_=WTd.ap())

        ones16 = constp.tile([128, 128], F16, name="ones16")
        nc.vector.memset(ones16, 1.0)

        # ---- all input DMA triggers up front on the sync HWDGE ring ----
        embTs = []
        for b in range(BPC):
            embT = embT_p.tile([128, NDT * S], F16, name="embT")
            embTs.append(embT)
            dst = embT.rearrange("p (k s) -> p k s", k=NDT)
            src = embPd.ap()[b]
            k0 = 0
            for ck in CHUNKS[b]:
                nc.sync.dma_start(
                    out=dst[:, k0 : k0 + ck, :], in_=src[:, k0 : k0 + ck, :]
                )
                k0 += ck

        # ---- PE warm-up: keep the HAM clock-gate open while waiting for
        # the first input bytes. Results go to a scratch PSUM bank that is
        # never read.
        warm_ps = warmps_p.tile([128, 128], F32, name="warm_ps")
        for _ in range(N_WARM):
            nc.tensor.matmul(warm_ps, ones16, ones16, start=True, stop=True)

        def proj_alloc():
            projT = projT_p.tile([128, S], F16, name="projT")
            sq = sq_p.tile([128, S], F16, name="sq")
            pps = [projps_p.tile([128, 512], F32, name="projps") for _ in range(2)]
            return projT, sq, pps

        def proj_chunk(b, tiles, k0, k1):
            """Accumulating proj matmuls for k-blocks [k0, k1)."""
            embT = embTs[b]
            projT, sq, pps = tiles
            for k in range(k0, k1):
                for h in range(2):
                    nc.tensor.matmul(
                        pps[h],
                        WT16[:, 128 * k : 128 * (k + 1)],
                        embT[:, S * k + 512 * h : S * k + 512 * (h + 1)],
                        start=(k == 0),
                        stop=(k == NDT - 1),
                    )
            if k1 == NDT:
                for h in range(2):
                    sl = slice(512 * h, 512 * (h + 1))
                    nc.vector.tensor_copy(projT[:, sl], pps[h])
                    nc.vector.tensor_mul(sq[:, sl], projT[:, sl], projT[:, sl])

        def norms_batch(sq):
            """ncol f32 [128, 2/i-tile] (n_i per-partition) and
            rowrep fp16 [128, S] (n_j on every partition)."""
            ncol_ps = normps_p.tile([128, 16], F32, tag="np", name="ncol_ps")
            for i in range(NST):
                nc.tensor.matmul(
                    ncol_ps[:, 2 * i : 2 * i + 2],
                    sq[:, 128 * i : 128 * (i + 1)],
                    ones16[:, 0:2],
                    start=True,
                    stop=True,
                )
            ncol = ncol_p.tile([128, 2 * NST], F32, name="ncol")
            nc.vector.tensor_copy(ncol, ncol_ps)

            rowrep = rowrep_p.tile([128, S], F16, name="rowrep")
            for h in range(2):
                rp = normps_p.tile([128, 512], F32, tag="np", name="rp_ps")
                nc.tensor.matmul(
                    rp, ones16, sq[:, 512 * h : 512 * (h + 1)],
                    start=True, stop=True,
                )
                nc.scalar.copy(rowrep[:, 512 * h : 512 * (h + 1)], rp)
            return ncol, rowrep

        def dots_tile(b, i, outsb, projT, ncol, rowrep, nchunk, no_gps=False):
            j0 = 128 * i
            Wi = WIDTHS[i]
            off = OFFS[i]
            pos = 0
            while pos < Wi:
                w = min(512, Wi - pos)
                d_ps = dotps_p.tile([128, w], F32, tag="dp", name="d_ps")
                nc.tensor.matmul(
                    d_ps,
                    projT[:, j0 : j0 + 128],
                    projT[:, j0 + pos : j0 + pos + w],
                    start=True,
                    stop=True,
                )
                o = outsb[:, off + pos : off + pos + w]
                rr = rowrep[:, j0 + pos : j0 + pos + w]
                nb = ncol[:, 2 * i : 2 * i + 1]
                tmp = tmp_p.tile([128, 512], F16, name="tmp")[:, 0:w]
                # -2*dot + n_i: spread across ACT / DVE
                if nchunk % 3 == 2:
                    nc.vector.tensor_scalar(tmp, d_ps, -2.0, nb, MULT, ADD)
                else:
                    nc.scalar.activation(tmp, d_ps, IDENT, bias=nb, scale=-2.0)
                # + n_j: spread DVE / GPSIMD
                if nchunk % 2 == 0 or no_gps:
                    nc.vector.tensor_add(o, tmp, rr)
                else:
                    nc.gpsimd.tensor_add(o, tmp, rr)
                nchunk += 1
                pos += w
            return nchunk

        # ---- main pipeline ----
        # batch 0 proj follows its input chunks; later batches' proj
        # chunks are interleaved into the previous batch's dots stream.
        tiles = proj_alloc()
        k0 = 0
        for ck in CHUNKS[0]:
            proj_chunk(0, tiles, k0, k0 + ck)
            k0 += ck

        for b in range(BPC):
            last = b + 1 >= BPC
            projT, sq, _ = tiles
            ncol, rowrep = norms_batch(sq)
            outsb = out_p.tile([128, TOT], F16, name="outsb")
            nchunk = 0
            cut = 0
            for i in range(NST):
                nchunk = dots_tile(
                    b, i, outsb, projT, ncol, rowrep, nchunk,
                    no_gps=(last and i >= 4),
                )
                if cut < len(OUT_CUTS) and OUT_CUTS[cut][0] == i:
                    _, c0, c1 = OUT_CUTS[cut]
                    nc.sync.dma_start(
                        out=outPd.ap()[b, :, c0:c1], in_=outsb[:, c0:c1]
                    )
                    cut += 1
                if not last:
                    # interleave next batch's proj after tiles 3 and 5
                    if i == 3:
                        tiles_n = proj_alloc()
                        proj_chunk(b + 1, tiles_n, 0, CHUNKS[b + 1][0])
                    elif i == 5:
                        proj_chunk(b + 1, tiles_n, CHUNKS[b + 1][0], NDT)
            if not last:
                tiles = tiles_n

    nc.finalize()
    return nc


_NC_CACHE = None


def _get_nc():
    global _NC_CACHE
    if _NC_CACHE is None:
        _NC_CACHE = build_nc()
    return _NC_CACHE


def _host_wt16(W):
    # WT16[p, 128k + j] = W[j, 128k + p]  (W^T in [d-part, k, r] blocks)
    Wf = np.asarray(W, dtype=np.float32)
    wt = Wf.T.reshape(NDT, 128, 128).transpose(1, 0, 2).reshape(128, D)
    return np.ascontiguousarray(wt).astype(np.float16)


def _host_embp(emb16_core):
    # embP[b, p, k, s] = emb16[b, s, 128k + p]
    return np.ascontiguousarray(
        emb16_core.reshape(BPC, S, NDT, 128).transpose(0, 3, 2, 1)
    )


def run(embeddings_batch, W, trace=False, tmpdir=None):
    nc = _get_nc()
    emb16 = np.asarray(embeddings_batch, dtype=np.float32).astype(np.float16)
    wt16 = _host_wt16(W)
    in_maps = [
        {
            "embP16": _host_embp(emb16[c * BPC : (c + 1) * BPC]),
            "WT16": wt16,
        }
        for c in range(NCORES)
    ]
    res = run_bass_kernel_spmd(
        nc, in_maps, core_ids=list(range(NCORES)), trace=trace, tmpdir=tmpdir
    )
    # unpack: outP16 [BPC, 128, TOT] -> dist blocks j >= i; mirror the rest
    full = np.empty((B, S, S), dtype=np.float16)
    for c in range(NCORES):
        P = res.results[c]["outP16"]
        for b in range(BPC):
            g = c * BPC + b
            for i in range(NST):
                full[g, 128 * i : 128 * (i + 1), 128 * i : S] = P[
                    b, :, OFFS[i] : OFFS[i] + WIDTHS[i]
                ]
    NB = NST
    M = full.reshape(B, NB, 128, NB, 128)
    iu = np.triu_indices(NB, 1)
    M[:, iu[1], :, iu[0], :] = M[:, iu[0], :, iu[1], :].swapaxes(-1, -2)
    return full.astype(np.float32), res


def kernel(embeddings_batch, W):
    full, _ = run(embeddings_batch, W, trace=False)
    return full
